# revision 37
# baseline (speedup 1.0000x reference)
"""Causal self-attention (B=4, T=2048, C=1024, H=16) on 8 TRN2 NeuronCores.

Sharding: 8 cores = 4 batches x 2 head-groups (8 heads each). Core c = g*4+b
handles batch b, heads 8g..8g+8 (4 pairs of 2). Host transposes x[b] -> xT
[C,T] in bf16, slices W_attn columns (Wq pre-scaled by 1/sqrt(D)) and W_proj
rows per group (bf16), runs one Bass/Tile kernel SPMD on cores 0-7, then sums
the two group-partial out^T [C,T] (bf16) per batch and transposes to f32.

Per-core device pipeline (all matmul inputs bf16, PSUM f32):
  A. QK^T projection for pair 0 (weight-stationary over T/2-half PSUM accs)
     with V-projection tiles (natural layout + fused ones column) interleaved.
  B. attention per (pair, head, T/2 query-half), software-pipelined over key
     tiles jt: scores^T = K_jt @ Q^T (<=512 bank-aligned segs) -> one ACT exp
     per jt -> es bf16 -> DVE mask-mul on the diagonal -> PV [V|1]^T @ es
     accumulating y^T+sums [65,1024] PSUM, trailing one jt so ACT (the phase
     bottleneck) never stalls; completed 512-strips evacuate to SBUF at once.
     Normalize per half: DVE recip, Pool partition_broadcast, DVE mul -> bf16.
     The NEXT pair's QK projection is drip-fed between jt steps as 2-matmul
     quanta so the PE stays busy under the ACT-bound phase.
  C. output projection, weight-stationary, PSUM slots rotated across the
     freed pools; copies alternate DVE/ACT; DMA out bf16 on the SP queue.
"""
import sys
if '/opt/trn_rl_repo' not in sys.path:
    sys.path.insert(0, '/opt/trn_rl_repo')
import collections
import numpy as np
import ml_dtypes
import concourse.bacc as bacc
import concourse.tile as tile
import concourse.mybir as mybir
from concourse import bass_utils

F32 = mybir.dt.float32
BF16 = mybir.dt.bfloat16
EXP = mybir.ActivationFunctionType.Exp

N_EMBED = 1024
N_HEAD = 16
D = 64
B_FULL, T_FULL, C_FULL = 4, 2048, 1024
N_GROUPS = 2


def build_kernel(T=T_FULL, C=C_FULL, n_pairs=4, reps=1, out_dt=BF16,
                 interleave=True):
    HP = n_pairs * 2          # heads per core
    CIN = HP * D              # 512
    n_k = C // 128            # contraction tiles
    n_jt = T // 128           # key tiles
    HT = T // 2               # query half

    nc = bacc.Bacc("TRN2", target_bir_lowering=False, debug=False)
    xt_d = nc.dram_tensor("xt", [C, T], BF16, kind="ExternalInput")
    wqk_d = nc.dram_tensor("wqk", [C, n_pairs * 2 * 128], BF16, kind="ExternalInput")
    wv_d = nc.dram_tensor("wv", [C, n_pairs * 128], BF16, kind="ExternalInput")
    wp_d = nc.dram_tensor("wp", [CIN, C], BF16, kind="ExternalInput")
    mask_d = nc.dram_tensor("mask", [128, 128], F32, kind="ExternalInput")
    outp_d = nc.dram_tensor("outp", [C, T], out_dt, kind="ExternalOutput")

    xt_r = xt_d.ap().rearrange("(k p) t -> p k t", p=128)
    wqk_r = wqk_d.ap().rearrange("(k p) m -> p k m", p=128)
    wv_r = wv_d.ap().rearrange("(k p) m -> p k m", p=128)
    wp_r = wp_d.ap().rearrange("(k p) m -> p k m", p=128)

    with tile.TileContext(nc) as tc:
        with tc.tile_pool(name="wts", bufs=1) as wts, \
             tc.tile_pool(name="xsp", bufs=1) as xsp, \
             tc.tile_pool(name="qkp", bufs=1) as qkp, \
             tc.tile_pool(name="vp", bufs=1) as vp, \
             tc.tile_pool(name="esp", bufs=5) as esp, \
             tc.tile_pool(name="normp", bufs=2) as normp, \
             tc.tile_pool(name="ysbp", bufs=1) as ysbp, \
             tc.tile_pool(name="osbp", bufs=4) as osbp, \
             tc.tile_pool(name="ps_y", bufs=1, space="PSUM") as ps_y, \
             tc.tile_pool(name="ps_a", bufs=2, space="PSUM") as ps_a, \
             tc.tile_pool(name="ps_qk", bufs=1, space="PSUM") as ps_qk:

            def body(_i=None):
                # ------------- DMAs (SP: weights; ACT queue: x) -------------
                # SP queue: pair-0 weights, wv, late x strips, rest of wqk.
                # ACT queue: early x strips. First matmul can start ~2.6us.
                wqk_sb = wts.tile([128, n_k, n_pairs * 2 * 128], BF16, tag="wqk")
                wv_sb = wts.tile([128, n_k, n_pairs * 128], BF16, tag="wv")
                mask_sb = wts.tile([128, 128], F32, tag="mask")
                wp_sb = wts.tile([128, CIN // 128, C], BF16, tag="wp")
                xs = xsp.tile([128, n_k, T], BF16, tag="xs")
                nc.sync.dma_start(out=wqk_sb[:, :, 0:256], in_=wqk_r[:, :, 0:256])
                for s in range(2):
                    sl = slice(s * 512, (s + 1) * 512)
                    nc.scalar.dma_start(out=xs[:, :, sl], in_=xt_r[:, :, sl])
                for s in range(2, T // 512):
                    sl = slice(s * 512, (s + 1) * 512)
                    nc.sync.dma_start(out=xs[:, :, sl], in_=xt_r[:, :, sl])
                nc.sync.dma_start(out=wv_sb[:], in_=wv_r)
                nc.sync.dma_start(out=wqk_sb[:, :, 256:], in_=wqk_r[:, :, 256:])
                nc.sync.dma_start(out=mask_sb[:], in_=mask_d.ap())
                nc.sync.dma_start(out=wp_sb[:], in_=wp_r)

                qt = [qkp.tile([128, T], BF16, tag=f"qt{p}", name=f"qt{p}")
                      for p in range(n_pairs)]
                kt = [qkp.tile([128, T], BF16, tag=f"kt{p}", name=f"kt{p}")
                      for p in range(n_pairs)]
                v_aug = vp.tile([128, n_jt, HP, 65], BF16)
                nc.vector.memset(v_aug[:, :, :, 64:65], 1.0)

                # ------------- projection emitters --------------------------
                def v_tile(nt, copy_fn, pool=None, tag="a"):
                    psv = (pool or ps_a).tile(
                        [128, n_pairs * 128], F32, tag=tag, name=f"psv{nt}")
                    for k in range(n_k):
                        nc.tensor.matmul(
                            psv[:], xs[:, k, nt * 128:(nt + 1) * 128],
                            wv_sb[:, k, :], start=(k == 0), stop=(k == n_k - 1))
                    copy_fn(
                        v_aug[:, nt, :, 0:64],
                        psv[:].rearrange("q (h d) -> q h d", d=D))

                def qk_group(p, qk, half, pool, tag):
                    """Emit one (qk,half) group now, strip-major so the first
                    matmuls only need the earliest x strips off the wire."""
                    i = p * 2 + qk
                    acc = pool.tile([128, HT], F32, tag=tag,
                                    name=f"qkaccA{p}_{qk}_{half}")
                    for s in range(2):
                        col = half * HT + s * 512
                        for k in range(n_k):
                            nc.tensor.matmul(
                                acc[:, s * 512:(s + 1) * 512],
                                wqk_sb[:, k, i * 128:(i + 1) * 128],
                                xs[:, k, col:col + 512],
                                start=(k == 0), stop=(k == n_k - 1),
                                skip_group_check=True)
                    dst = (qt if qk == 0 else kt)[p]
                    nc.vector.tensor_copy(
                        dst[:, half * HT:(half + 1) * HT], acc[:])

                def qk_group_quanta(p, qk, half):
                    """Yield 2-matmul k-step closures for one (qk,half) group."""
                    i = p * 2 + qk
                    box = {}

                    def step(k, box=box, p=p, qk=qk, half=half, i=i):
                        if k == 0:
                            box["acc"] = ps_qk.tile(
                                [128, HT], F32, tag="qkacc",
                                name=f"qkacc{p}_{qk}_{half}")
                        acc = box["acc"]
                        for s in range(2):
                            col = half * HT + s * 512
                            nc.tensor.matmul(
                                acc[:, s * 512:(s + 1) * 512],
                                wqk_sb[:, k, i * 128:(i + 1) * 128],
                                xs[:, k, col:col + 512],
                                start=(k == 0), stop=(k == n_k - 1),
                                skip_group_check=True)
                        if k == n_k - 1:
                            dst = (qt if qk == 0 else kt)[p]
                            nc.vector.tensor_copy(
                                dst[:, half * HT:(half + 1) * HT], acc[:])

                    for k in range(n_k):
                        yield (lambda k=k: step(k))

                # ------------- phase A: pair-0 QK + first-half V ------------
                # accumulators alternate between the two free PSUM slots so a
                # group never waits on the previous group's evacuation copy
                for g, (half, qk) in enumerate(
                        ((0, 0), (0, 1), (1, 0), (1, 1))):
                    pool, tag = ((ps_qk, "qkacc"), (ps_y, "y"))[g % 2]
                    qk_group(0, qk, half, pool, tag)
                for nt in range(n_jt // 2 if interleave else n_jt):
                    v_tile(nt, nc.scalar.copy if nt % 2 else nc.vector.tensor_copy)
                if not interleave:
                    for g, (half, qk) in enumerate(
                            ((0, 0), (0, 1), (1, 0), (1, 1))):
                        for p in range(1, n_pairs):
                            pool, tag = ((ps_qk, "qkacc"), (ps_y, "y"))[g % 2]
                            qk_group(p, qk, half, pool, tag)

                # ------------- phase B: attention ---------------------------
                # `pending` defers PV (and per-unit normalize) emission by ~3
                # jt steps, across unit boundaries, so the PE never meets a
                # just-issued exp; `quanta` drip-feeds the next pair's QK
                # projection between steps.
                quanta = collections.deque()
                pending = collections.deque()
                ysb = ysbp.tile([128, n_pairs, T], BF16)
                for p in range(n_pairs):
                    if p == 0 and interleave:
                        # late-half V tiles drip-feed first (consumed from u=1)
                        quanta.extend(
                            (lambda nt=nt: v_tile(
                                nt, nc.vector.tensor_copy, ps_qk, "qkacc"))
                            for nt in range(n_jt // 2, n_jt))
                    if interleave and p + 1 < n_pairs:
                        for qk in (0, 1):
                            for half in (0, 1):
                                quanta.extend(qk_group_quanta(p + 1, qk, half))
                    for h in range(2):
                        hh = p * 2 + h
                        hs = slice(h * 64, (h + 1) * 64)
                        for u in range(2):
                            qlo, qhi = u * HT, (u + 1) * HT
                            y_ps = ps_y.tile([65, HT], F32, tag="y")
                            y_sb = normp.tile([65, HT], F32, tag="ysb")

                            def emit_pv(jt, es, y_ps=y_ps, y_sb=y_sb,
                                        hh=hh, qlo=qlo, qhi=qhi):
                                lo = 128 * jt
                                a0 = max(lo, qlo)
                                pieces = []
                                a = a0
                                while a < qhi:
                                    e = min((a // 512 + 1) * 512, qhi)
                                    pieces.append((a, e))
                                    a = e
                                for (a, e) in pieces:
                                    nc.tensor.matmul(
                                        y_ps[:, a - qlo:e - qlo],
                                        v_aug[:, jt, hh, :],
                                        es[:, a - qlo:e - qlo],
                                        start=(jt == 0),
                                        stop=(jt == (a // 512) * 4 + 3),
                                        skip_group_check=True)
                                if jt % 4 == 3 and lo >= qlo:
                                    # diagonal strip complete: evacuate now
                                    s0 = lo // 512
                                    lsl = slice(s0 * 512 - qlo,
                                                (s0 + 1) * 512 - qlo)
                                    nc.vector.tensor_copy(
                                        y_sb[:, lsl], y_ps[:, lsl])

                            for jt in range(8 * (u + 1)):
                                lo = 128 * jt
                                a0 = max(lo, qlo)
                                es = esp.tile([128, HT], BF16, tag="es")
                                ps = ps_a.tile([128, HT], F32, tag="a")
                                a = a0
                                while a < qhi:
                                    e = min((a // 512 + 1) * 512, qhi)
                                    nc.tensor.matmul(
                                        ps[:, a - qlo:e - qlo],
                                        kt[p][hs, lo:lo + 128],
                                        qt[p][hs, a:e],
                                        start=True, stop=True)
                                    a = e
                                if lo >= qlo:
                                    # additive causal mask (-1e30) on the
                                    # diagonal block, pre-exp: PV then only
                                    # depends on exp, never on the DVE
                                    nc.vector.tensor_add(
                                        ps[:, lo - qlo:lo - qlo + 128],
                                        ps[:, lo - qlo:lo - qlo + 128],
                                        mask_sb[:])
                                nc.scalar.activation(
                                    out=es[:, a0 - qlo:HT],
                                    in_=ps[:, a0 - qlo:HT], func=EXP)
                                pending.append(
                                    lambda jt=jt, es=es, f=emit_pv: f(jt, es))
                                if len(pending) > 3:
                                    pending.popleft()()
                                if quanta:
                                    quanta.popleft()()

                            def norm(y_sb=y_sb, h=h, p=p, qlo=qlo, qhi=qhi):
                                # normalize this half in SBUF
                                recip = normp.tile([1, HT], F32, tag="recip")
                                nc.vector.reciprocal(recip[:], y_sb[64:65, :])
                                bcast = normp.tile([64, HT], F32, tag="bcast")
                                nc.gpsimd.partition_broadcast(
                                    bcast[:], recip[:])
                                nc.vector.tensor_mul(
                                    ysb[h * 64:(h + 1) * 64, p, qlo:qhi],
                                    y_sb[0:64, :], bcast[:])

                            pending.append(norm)
                while pending:
                    pending.popleft()()
                    if quanta:
                        quanta.popleft()()
                while quanta:
                    quanta.popleft()()

                # ------------- phase C: output projection -------------------
                pso_src = [(ps_a, "a"), (ps_a, "a"), (ps_qk, "qkacc"),
                           (ps_y, "y")]
                for half in range(2):
                    sl = slice(half * HT, (half + 1) * HT)
                    for m in range(C // 128):
                        pool, tag = pso_src[m % 4]
                        pso = pool.tile([128, HT], F32, tag=tag, name=f"pso{m}")
                        for kp in range(CIN // 128):
                            for s in range(2):
                                nc.tensor.matmul(
                                    pso[:, s * 512:(s + 1) * 512],
                                    wp_sb[:, kp, m * 128:(m + 1) * 128],
                                    ysb[:, kp, half * HT + s * 512:
                                        half * HT + (s + 1) * 512],
                                    start=(kp == 0), stop=(kp == CIN // 128 - 1),
                                    skip_group_check=True)
                        osb = osbp.tile([128, HT], out_dt, tag="osb")
                        (nc.vector.tensor_copy if m % 2 == 0 else nc.scalar.copy)(
                            osb[:], pso[:])
                        nc.sync.dma_start(
                            out=outp_d.ap()[m * 128:(m + 1) * 128, sl], in_=osb[:])

            if reps == 1:
                body()
            else:
                with tc.For_i(0, reps, 1) as i:
                    body(i)
    nc.compile()
    return nc


def host_inputs(x, W_attn, W_proj, n_groups=N_GROUPS):
    """Per-core input maps (bf16). Core order: g * B + b."""
    B, T, C = x.shape
    hp = N_HEAD // n_groups
    n_pairs = hp // 2
    bf16 = ml_dtypes.bfloat16
    scale = np.float32(1.0 / np.sqrt(D))
    allowed = np.arange(128)[None, :] >= np.arange(128)[:, None]
    mask = np.where(allowed, 0.0, -1e30).astype(np.float32)
    in_maps = []
    for g in range(n_groups):
        qk_cols, v_cols = [], []
        for p in range(n_pairs):
            h0 = g * hp + 2 * p
            h1 = h0 + 1
            qk_cols.append(W_attn[:, h0 * D:(h0 + 1) * D] * scale)
            qk_cols.append(W_attn[:, h1 * D:(h1 + 1) * D] * scale)
            qk_cols.append(W_attn[:, C + h0 * D:C + (h0 + 1) * D])
            qk_cols.append(W_attn[:, C + h1 * D:C + (h1 + 1) * D])
            v_cols.append(W_attn[:, 2 * C + h0 * D:2 * C + (h0 + 1) * D])
            v_cols.append(W_attn[:, 2 * C + h1 * D:2 * C + (h1 + 1) * D])
        wqk = np.ascontiguousarray(
            np.concatenate(qk_cols, axis=1)).astype(bf16)
        wv = np.ascontiguousarray(np.concatenate(v_cols, axis=1)).astype(bf16)
        wp = np.ascontiguousarray(
            W_proj[g * hp * D:(g + 1) * hp * D]).astype(bf16)
        for b in range(B):
            xt = np.ascontiguousarray(x[b].T).astype(bf16)
            in_maps.append({"xt": xt, "wqk": wqk, "wv": wv, "wp": wp,
                            "mask": mask})
    return in_maps


def host_gather(results, B, T, C, n_groups=N_GROUPS):
    out = np.zeros((B, T, C), dtype=np.float32)
    for g in range(n_groups):
        for b in range(B):
            out[b] += results[g * B + b]["outp"].astype(np.float32).T
    return out


_NC_CACHE = {}


def kernel(x, W_attn, W_proj):
    x = np.asarray(x, dtype=np.float32)
    W_attn = np.asarray(W_attn, dtype=np.float32)
    W_proj = np.asarray(W_proj, dtype=np.float32)
    B, T, C = x.shape
    if "nc" not in _NC_CACHE:
        _NC_CACHE["nc"] = build_kernel(T=T, C=C)
    nc = _NC_CACHE["nc"]
    in_maps = host_inputs(x, W_attn, W_proj)
    res = bass_utils.run_bass_kernel_spmd(nc, in_maps, core_ids=list(range(8)))
    return host_gather(res.results, B, T, C)
